# revision 39
# baseline (speedup 1.0000x reference)
"""Trainium2 Bass kernel for AttentionSummarization (segment_reduce).

Pipeline (reference semantics):
  1. spatial attention over R=49 regions -> fbar [T, D]
  2. per-segment mean pooling over T=4096 frames into S=256 segments
  3. FCSN: conv1d(k=3) + relu + conv1d(k=1) over segments -> keysteps [106, S]
  4. softmax over classes, keystep-weighted segment reduce -> f_keysteps
  5. keystep attention -> f_videos -> category logits

Sharding: T=4096 frames split across 8 cores (512 frames each). Each core
streams its 51.4MB feature shard once (memory-bound part), computes spatial
attention + per-segment partial sums on device, AllReduces the [256, 512]
partial segment sums, then every core redundantly computes the tiny FCSN /
attention tail; core 0's tail outputs are used.

Inner loop, per chunk of 7 regions ([128, 7, 512], 1.75MB fp32 HBM read):
  - DMA : gpsimd SWDGE cast-DMA loads X as fp16 (HBM still reads the
          full fp32 bytes, SBUF holds half) — this path also starts
          before the collective-runtime entry barrier, hiding launch skew
  - DVE : one batched fp16 multiply prod = X * att_w at 2x mode (~2us),
          a batched free-dim reduce for 4 regions, three diag builds
  - ACT : copy+accum_out dot reductions for 3 regions, batched exp,
          four diag(e) = identity * e builds
  - PE  : psum += diag(e_r) @ X_r per region (fp16 matmul, fp32 PSUM)
The loop is DMA-bound (~270GB/s on the cast path); fp16 keeps rel err
at ~4e-4 (fp16 mantissa, 11 bits, vs bf16's 8).
Segment partial sums accumulate in two pinned PSUM banks via one-hot
matmuls (float32r); the [256, 512] partials are AllReduced across the
8 cores and the FCSN tail runs redundantly on every core in float32r
(producers round via bitcast).
Softmax is computed without max-subtraction: s = feature . att_w has
sigma ~ 0.45, so exp() is numerically safe; the additive att_b/att2_b
biases are softmax-invariant and dropped.
"""

import os

os.environ.setdefault("MYCRO_LOCAL_CACHE", "1")

import numpy as np

N_CORES = 8
T, R, D = 4096, 49, 512
TL = T // N_CORES          # frames per core = 512
NG = TL // 128             # groups of 128 frames = 4
RC = 7                     # regions per DMA chunk (49 = 7*7)
NCH = R // RC              # chunks per group = 7
NSEG = 256
NCLS, NCAT = 106, 18
F32 = np.float32

_BUILT = None  # cached (nc, out_names)


def _build_bass():
    """Build the Bass module (single SPMD program for all 8 cores)."""
    import concourse.bacc as bacc
    import concourse.mybir as mybir
    import concourse.tile as tile

    f32 = mybir.dt.float32
    f32r = mybir.dt.float32r
    bf16 = mybir.dt.float16
    Alu = mybir.AluOpType
    Act = mybir.ActivationFunctionType
    Ax = mybir.AxisListType

    nc = bacc.Bacc(
        "TRN2",
        debug=False,
        enable_asserts=False,
        target_bir_lowering=False,
        num_devices=N_CORES,
    )

    # ---- external inputs ----
    feat = nc.dram_tensor("feat", [TL, R, D], f32, kind="ExternalInput").ap()
    idsf = nc.dram_tensor("idsf", [NG, 128, 1], f32, kind="ExternalInput").ap()
    invc = nc.dram_tensor("invc", [128, 2], f32, kind="ExternalInput").ap()
    wbatt = nc.dram_tensor("wbatt", [128, D], f32, kind="ExternalInput").ap()
    iota = nc.dram_tensor("iota", [128, NSEG], f32, kind="ExternalInput").ap()
    ident = nc.dram_tensor("ident", [128, 128], f32, kind="ExternalInput").ap()
    w1t = nc.dram_tensor("w1t", [12, 128, D], f32, kind="ExternalInput").ap()
    b1p = nc.dram_tensor("b1p", [128, 4], f32, kind="ExternalInput").ap()
    w2t = nc.dram_tensor("w2t", [4, 128, NCLS], f32, kind="ExternalInput").ap()
    b2r = nc.dram_tensor("b2r", [128, NCLS], f32, kind="ExternalInput").ap()
    att2p = nc.dram_tensor("att2p", [128, 4], f32, kind="ExternalInput").ap()
    clswp = nc.dram_tensor("clswp", [128, 4 * NCAT], f32, kind="ExternalInput").ap()
    clsb = nc.dram_tensor("clsb", [1, NCAT], f32, kind="ExternalInput").ap()
    ones11 = nc.dram_tensor("ones11", [1, 1], f32, kind="ExternalInput").ap()

    # ---- external outputs ----
    alpha_out = nc.dram_tensor("alpha_out", [TL, R], f32, kind="ExternalOutput").ap()
    ksT_out = nc.dram_tensor("ksT_out", [NSEG, NCLS], f32, kind="ExternalOutput").ap()
    cats_out = nc.dram_tensor("cats_out", [1, NCAT], f32, kind="ExternalOutput").ap()
    alpha2_out = nc.dram_tensor("alpha2_out", [1, NCLS], f32, kind="ExternalOutput").ap()

    with tile.TileContext(nc) as tc:
        with (
            tc.tile_pool(name="const", bufs=1) as cpool,
            tc.tile_pool(name="xs", bufs=6) as xpool,
            tc.tile_pool(name="work", bufs=2) as work,
            tc.tile_pool(name="prodp", bufs=4) as prodp,
            tc.tile_pool(name="diagp", bufs=10) as diagp,
            tc.tile_pool(name="small", bufs=5) as small,
            tc.tile_pool(name="mainps", bufs=2, space="PSUM") as mainps,
            tc.tile_pool(name="segps", bufs=1, space="PSUM") as segps,
            tc.tile_pool(name="dram", bufs=1, space="DRAM") as dram,
        ):
            # ---- main-loop constants into SBUF (tail weights loaded later) ----
            wb7 = cpool.tile([128, RC, D], bf16)
            for j in range(RC):
                nc.gpsimd.dma_start(wb7[:, j, :], wbatt)
            iot = cpool.tile([128, NSEG], f32)
            nc.gpsimd.dma_start(iot[:], iota)
            idn = cpool.tile([128, 128], f32)
            nc.gpsimd.dma_start(idn[:], ident)
            idnh = cpool.tile([128, 128], bf16)
            nc.gpsimd.dma_start(idnh[:], ident)

            # ---- pinned per-core segment-sum accumulators (PSUM) ----
            segA = segps.tile([128, D], f32, name="segA")
            segB = segps.tile([128, D], f32, name="segB")

            # ================= main loop: spatial attention =================
            for g in range(NG):
                nump = mainps.tile([128, D], f32, name="nump")
                E = small.tile([128, R], f32, name="E")
                for c in range(NCH):
                    X = xpool.tile([128, RC, D], bf16, name="X")
                    nc.gpsimd.dma_start(
                        X[:], feat[g * 128:(g + 1) * 128, c * RC:(c + 1) * RC, :]
                    )
                    # one batched multiply for all RC regions of this chunk
                    prod = prodp.tile([128, RC, D], bf16, name="prod")
                    nc.vector.tensor_tensor(
                        out=prod[:], in0=X[:], in1=wb7[:], op=Alu.mult
                    )
                    # dot reductions: rr=0..3 one batched DVE reduce, rr=4..6 ACT
                    S7 = small.tile([128, RC], f32, name="S7")
                    nc.vector.reduce_sum(S7[:, 0:4], prod[:, 0:4, :], axis=Ax.X)
                    for rr in range(4, RC):
                        scrap = work.tile([128, D], bf16, name="scrap")
                        nc.scalar.activation(
                            scrap[:], prod[:, rr, :], Act.Copy,
                            accum_out=S7[:, rr:rr + 1],
                        )
                    # batched exp for the chunk
                    nc.scalar.activation(
                        E[:, c * RC:(c + 1) * RC], S7[:], Act.Exp
                    )
                    for rr in range(RC):
                        r = c * RC + rr
                        diag = diagp.tile([128, 128], bf16, name="diag")
                        if rr < 3:
                            nc.vector.tensor_scalar_mul(
                                diag[:], idnh[:], E[:, r:r + 1]
                            )
                        else:
                            nc.scalar.activation(
                                diag[:], idnh[:], Act.Copy, scale=E[:, r:r + 1]
                            )
                        nc.tensor.matmul(
                            out=nump[:],
                            lhsT=diag[:],
                            rhs=X[:, rr, :],
                            start=(r == 0),
                            stop=(r == R - 1),
                        )
                # ---- group finalize ----
                den = small.tile([128, 1], f32, name="den")
                nc.vector.reduce_sum(den[:], E[:], axis=Ax.X)
                rec = small.tile([128, 1], f32, name="rec")
                nc.vector.reciprocal(rec[:], den[:])
                alph = small.tile([128, R], f32, name="alph")
                nc.scalar.activation(alph[:], E[:], Act.Copy, scale=rec[:])
                nc.sync.dma_start(alpha_out[g * 128:(g + 1) * 128, :], alph[:])
                fbar = work.tile([128, D], f32, name="fbar")
                nc.scalar.activation(fbar[:].bitcast(f32r), nump[:], Act.Copy, scale=rec[:])
                idc = small.tile([128, 1], f32, name="idc")
                nc.gpsimd.dma_start(idc[:], idsf[g])
                oh = work.tile([128, NSEG], f32, name="oh")
                nc.vector.tensor_scalar(
                    out=oh[:].bitcast(f32r), in0=iot[:], scalar1=idc[:], scalar2=None,
                    op0=Alu.is_equal,
                )
                nc.tensor.matmul(
                    out=segA[:], lhsT=oh[:, 0:128].bitcast(f32r), rhs=fbar[:].bitcast(f32r),
                    start=(g == 0), stop=(g == NG - 1), skip_group_check=True,
                )
                nc.tensor.matmul(
                    out=segB[:], lhsT=oh[:, 128:256].bitcast(f32r), rhs=fbar[:].bitcast(f32r),
                    start=(g == 0), stop=(g == NG - 1), skip_group_check=True,
                )

            # ============ cross-core segment-sum AllReduce ============
            segAs = work.tile([128, D], f32, name="segAs")
            nc.scalar.activation(segAs[:], segA[:], Act.Copy)
            segBs = work.tile([128, D], f32, name="segBs")
            nc.scalar.activation(segBs[:], segB[:], Act.Copy)
            ccin = dram.tile([NSEG, D], f32, name="ccin")
            nc.sync.dma_start(ccin[0:128, :], segAs[:])
            nc.sync.dma_start(ccin[128:256, :], segBs[:])
            ccout = dram.tile([NSEG, D], f32, name="ccout")
            nc.gpsimd.collective_compute(
                "AllReduce",
                Alu.add,
                replica_groups=[list(range(N_CORES))],
                ins=[ccin[:].opt()],
                outs=[ccout[:].opt()],
            )

            # ---- tail weights: load + f32r-round (overlaps the collective) ----
            w1sb = cpool.tile([128, 12, D], f32)
            for kc in range(12):
                wslot = work.tile([128, D], f32, name="wslot")
                nc.gpsimd.dma_start(wslot[:], w1t[kc])
                nc.scalar.activation(
                    w1sb[:, kc, :].bitcast(f32r), wslot[:], Act.Copy
                )
            b1sb = cpool.tile([128, 4], f32)
            nc.gpsimd.dma_start(b1sb[:], b1p)
            w2sb = cpool.tile([128, 4, NCLS], f32)
            for ic in range(4):
                wslot = work.tile([128, D], f32, name="wslot")
                nc.gpsimd.dma_start(wslot[:, 0:NCLS], w2t[ic])
                nc.scalar.activation(
                    w2sb[:, ic, :].bitcast(f32r), wslot[:, 0:NCLS], Act.Copy
                )
            b2sb = cpool.tile([128, NCLS], f32)
            nc.gpsimd.dma_start(b2sb[:], b2r)
            a2sb = cpool.tile([128, 4], f32)
            nc.gpsimd.dma_start(a2sb[:], att2p)
            clssb = cpool.tile([128, 4 * NCAT], f32)
            nc.gpsimd.dma_start(clssb[:], clswp)
            clsbsb = cpool.tile([1, NCAT], f32)
            nc.gpsimd.dma_start(clsbsb[:], clsb)
            onesb = cpool.tile([1, 1], f32)
            nc.gpsimd.dma_start(onesb[:], ones11)
            invsb = cpool.tile([128, 2], f32)
            nc.gpsimd.dma_start(invsb[:], invc)



            # ============ tail: segment means + FCSN + attention ============
            # fseg[s, d] (segment means), two 128-seg tiles
            fsegs = []
            for j in range(2):
                raw = work.tile([128, D], f32, name="fsraw")
                nc.sync.dma_start(raw[:], ccout[j * 128:(j + 1) * 128, :])
                fs = cpool.tile([128, D], f32, name=f"fseg{j}", tag=f"fseg{j}")
                nc.scalar.activation(fs[:].bitcast(f32r), raw[:], Act.Copy, scale=invsb[:, j:j + 1])
                fsegs.append(fs)

            # fsegT[d, s] with one zero column of padding on each side
            fsegT = []
            for jd in range(4):
                ft = cpool.tile([128, NSEG + 2], f32, name=f"fsegT{jd}", tag=f"fsegT{jd}")
                nc.scalar.activation(ft[:, 0:1].bitcast(f32r), idn[:, 0:1], Act.Copy, scale=0.0)
                nc.scalar.activation(ft[:, NSEG + 1:NSEG + 2].bitcast(f32r), idn[:, 0:1], Act.Copy, scale=0.0)
                for j in range(2):
                    tp = mainps.tile([128, 128], f32, name="tp")
                    nc.tensor.transpose(
                        tp[:], fsegs[j][:, jd * 128:(jd + 1) * 128], idn[:]
                    )
                    nc.scalar.activation(
                        ft[:, 1 + j * 128:1 + (j + 1) * 128].bitcast(f32r),
                        tp[:], Act.Copy
                    )
                fsegT.append(ft)

            # conv1 (k=3, SAME) + relu -> h[o, s], 4 o-tiles
            hts = []
            for ot in range(4):
                hps = mainps.tile([128, NSEG], f32, name="hps")
                first = True
                for k in range(3):
                    for icn in range(4):
                        nc.tensor.matmul(
                            out=hps[:],
                            lhsT=w1sb[:, k * 4 + icn, ot * 128:(ot + 1) * 128].bitcast(f32r),
                            rhs=fsegT[icn][:, k:k + NSEG].bitcast(f32r),
                            start=first,
                            stop=(k == 2 and icn == 3),
                        )
                        first = False
                ht = cpool.tile([128, NSEG], f32, name=f"h{ot}", tag=f"h{ot}")
                nc.scalar.activation(
                    ht[:].bitcast(f32r), hps[:], Act.Relu, bias=b1sb[:, ot:ot + 1]
                )
                hts.append(ht)

            # conv2 (k=1) -> keysteps^T [s, m]; + bias; softmax over m
            attT = []
            for st in range(2):
                kps = mainps.tile([128, NCLS], f32, name="kps")
                for icn in range(4):
                    nc.tensor.matmul(
                        out=kps[:],
                        lhsT=hts[icn][:, st * 128:(st + 1) * 128].bitcast(f32r),
                        rhs=w2sb[:, icn, :].bitcast(f32r),
                        start=(icn == 0),
                        stop=(icn == 3),
                    )
                ksT = work.tile([128, NCLS], f32, name="ksT")
                nc.vector.tensor_tensor(
                    out=ksT[:], in0=kps[:], in1=b2sb[:], op=Alu.add
                )
                nc.sync.dma_start(ksT_out[st * 128:(st + 1) * 128, :], ksT[:])
                mx = small.tile([128, 1], f32, name="mx")
                nc.vector.reduce_max(mx[:], ksT[:], axis=Ax.X)
                nmx = small.tile([128, 1], f32, name="nmx")
                nc.scalar.activation(nmx[:], mx[:], Act.Copy, scale=-1.0)
                ex = work.tile([128, NCLS], f32, name="ex")
                nc.scalar.activation(ex[:], ksT[:], Act.Exp, bias=nmx[:])
                sm = small.tile([128, 1], f32, name="sm")
                nc.vector.reduce_sum(sm[:], ex[:], axis=Ax.X)
                rc2 = small.tile([128, 1], f32, name="rc2")
                nc.vector.reciprocal(rc2[:], sm[:])
                at = cpool.tile([128, NCLS], f32, name=f"attT{st}", tag=f"attT{st}")
                nc.scalar.activation(at[:].bitcast(f32r), ex[:], Act.Copy, scale=rc2[:])
                attT.append(at)

            # f_keysteps [m, d] and [d, m]
            fkps = mainps.tile([NCLS, D], f32, name="fkps")
            for st in range(2):
                nc.tensor.matmul(
                    out=fkps[:], lhsT=attT[st][:].bitcast(f32r), rhs=fsegs[st][:].bitcast(f32r),
                    start=(st == 0), stop=(st == 1),
                )
            fk = cpool.tile([NCLS, D], f32, name="fk", tag="fk")
            nc.scalar.activation(fk[:], fkps[:], Act.Copy)
            fkT = []
            for jd in range(4):
                fkTps = mainps.tile([128, NCLS], f32, name="fkTps")
                for st in range(2):
                    nc.tensor.matmul(
                        out=fkTps[:],
                        lhsT=fsegs[st][:, jd * 128:(jd + 1) * 128].bitcast(f32r),
                        rhs=attT[st][:].bitcast(f32r),
                        start=(st == 0),
                        stop=(st == 1),
                    )
                fkt = cpool.tile([128, NCLS], f32, name=f"fkT{jd}", tag=f"fkT{jd}")
                nc.scalar.activation(fkt[:], fkTps[:], Act.Copy)
                fkT.append(fkt)

            # s2 = f_keysteps @ att2_w -> [1, m]; softmax over m
            s2ps = mainps.tile([1, NCLS], f32, name="s2ps")
            for jd in range(4):
                nc.tensor.matmul(
                    out=s2ps[:], lhsT=a2sb[:, jd:jd + 1], rhs=fkT[jd][:],
                    start=(jd == 0), stop=(jd == 3),
                )
            mx2 = small.tile([1, 1], f32, name="mx2")
            nc.vector.reduce_max(mx2[:], s2ps[:], axis=Ax.X)
            nmx2 = small.tile([1, 1], f32, name="nmx2")
            nc.scalar.activation(nmx2[:], mx2[:], Act.Copy, scale=-1.0)
            e2 = small.tile([1, NCLS], f32, name="e2")
            nc.scalar.activation(e2[:], s2ps[:], Act.Exp, bias=nmx2[:])
            sm2 = small.tile([1, 1], f32, name="sm2")
            nc.vector.reduce_sum(sm2[:], e2[:], axis=Ax.X)
            rc3 = small.tile([1, 1], f32, name="rc3")
            nc.vector.reciprocal(rc3[:], sm2[:])
            a2n = small.tile([1, NCLS], f32, name="a2n")
            nc.scalar.activation(a2n[:], e2[:], Act.Copy, scale=rc3[:])
            nc.sync.dma_start(alpha2_out[:], a2n[:])

            # alpha2 as a column via PE (contract dim 1)
            a2cps = mainps.tile([NCLS, 1], f32, name="a2cps")
            nc.tensor.matmul(out=a2cps[:], lhsT=a2n[:], rhs=onesb[:])
            a2c = small.tile([NCLS, 1], f32, name="a2c")
            nc.scalar.activation(a2c[:], a2cps[:], Act.Copy)

            # f_videos[d] = sum_m f_keysteps[m, d] * alpha2[m]
            fv = cpool.tile([128, 4], f32, name="fv", tag="fv")
            for jd in range(4):
                fvps = mainps.tile([128, 1], f32, name="fvps")
                nc.tensor.matmul(
                    out=fvps[:], lhsT=fk[:, jd * 128:(jd + 1) * 128], rhs=a2c[:]
                )
                nc.scalar.activation(fv[:, jd:jd + 1], fvps[:], Act.Copy)

            # cats = f_videos @ cls_w + cls_b
            cps = mainps.tile([1, NCAT], f32, name="cps")
            for jd in range(4):
                nc.tensor.matmul(
                    out=cps[:],
                    lhsT=fv[:, jd:jd + 1],
                    rhs=clssb[:, jd * NCAT:(jd + 1) * NCAT],
                    start=(jd == 0),
                    stop=(jd == 3),
                )
            catsb = small.tile([1, NCAT], f32, name="catsb")
            nc.vector.tensor_tensor(
                out=catsb[:], in0=cps[:], in1=clsbsb[:], op=Alu.add
            )
            nc.sync.dma_start(cats_out[:], catsb[:])

    nc.finalize()
    return nc


def _get_bass():
    global _BUILT
    if _BUILT is None:
        _BUILT = _build_bass()
    return _BUILT


def _make_in_maps(inputs):
    feature = np.asarray(inputs["feature"], dtype=F32)
    seg_ids = np.asarray(inputs["seg_ids"])
    att_w = np.asarray(inputs["att_w"], dtype=F32)
    att2_w = np.asarray(inputs["att2_w"], dtype=F32)
    fcsn_w1 = np.asarray(inputs["fcsn_w1"], dtype=F32)
    fcsn_b1 = np.asarray(inputs["fcsn_b1"], dtype=F32)
    fcsn_w2 = np.asarray(inputs["fcsn_w2"], dtype=F32)
    fcsn_b2 = np.asarray(inputs["fcsn_b2"], dtype=F32)
    cls_w = np.asarray(inputs["cls_w"], dtype=F32)
    cls_b = np.asarray(inputs["cls_b"], dtype=F32)

    ids = seg_ids.reshape(T).astype(np.int64)
    cnt = np.bincount(ids, minlength=NSEG)[:NSEG]
    invcnt = (1.0 / np.maximum(cnt, 1)).astype(F32)

    shared = {
        "invc": np.ascontiguousarray(invcnt.reshape(2, 128).T),
        "wbatt": np.ascontiguousarray(np.tile(att_w[:, 0], (128, 1))),
        "iota": np.ascontiguousarray(
            np.tile(np.arange(NSEG, dtype=F32), (128, 1))
        ),
        "ident": np.eye(128, dtype=F32),
        # w1t[k*4+ic] = fcsn_w1[:, ic*128:(ic+1)*128, k].T  ([i, o] layout)
        "w1t": np.ascontiguousarray(
            fcsn_w1.transpose(2, 1, 0).reshape(3, 4, 128, D).reshape(12, 128, D)
        ),
        "b1p": np.ascontiguousarray(fcsn_b1.reshape(4, 128).T),
        "w2t": np.ascontiguousarray(fcsn_w2[:, :, 0].T.reshape(4, 128, NCLS)),
        "b2r": np.ascontiguousarray(np.tile(fcsn_b2, (128, 1))),
        "att2p": np.ascontiguousarray(att2_w[:, 0].reshape(4, 128).T),
        "clswp": np.ascontiguousarray(
            cls_w.reshape(4, 128, NCAT).transpose(1, 0, 2).reshape(128, 4 * NCAT)
        ),
        "clsb": np.ascontiguousarray(cls_b.reshape(1, NCAT)),
        "ones11": np.ones((1, 1), dtype=F32),
    }
    in_maps = []
    for cix in range(N_CORES):
        m = dict(shared)
        m["feat"] = np.ascontiguousarray(feature[0, cix * TL:(cix + 1) * TL])
        m["idsf"] = np.ascontiguousarray(
            ids[cix * TL:(cix + 1) * TL].astype(F32).reshape(NG, 128, 1)
        )
        in_maps.append(m)
    return in_maps


def _assemble(results):
    alphas_sp = np.concatenate(
        [results[c]["alpha_out"] for c in range(N_CORES)], axis=0
    ).reshape(1, T, R, 1)
    keysteps = np.ascontiguousarray(results[0]["ksT_out"].T).reshape(1, NCLS, NSEG)
    cats = results[0]["cats_out"].reshape(1, NCAT)
    alphas_ks = results[0]["alpha2_out"].reshape(1, NCLS, 1)
    return (
        keysteps.astype(F32),
        cats.astype(F32),
        alphas_sp.astype(F32),
        alphas_ks.astype(F32),
    )


def run_kernel(inputs, trace=False, trace_kwargs=None):
    """Run on 8 NeuronCores. Returns (outputs_tuple, BassKernelResults)."""
    from concourse.bass_utils import run_bass_kernel_spmd

    nc = _get_bass()
    in_maps = _make_in_maps(inputs)
    kw = {}
    if trace:
        kw["trace"] = True
        if trace_kwargs:
            kw.update(trace_kwargs)
    rr = run_bass_kernel_spmd(nc, in_maps, core_ids=list(range(N_CORES)), **kw)
    return _assemble(rr.results), rr


def kernel(**inputs):
    outs, _ = run_kernel(inputs, trace=False)
    return outs


# revision 40
# speedup vs baseline: 1.1566x; 1.1566x over previous
"""Trainium2 Bass kernel for AttentionSummarization (segment_reduce).

Pipeline (reference semantics):
  1. spatial attention over R=49 regions -> fbar [T, D]
  2. per-segment mean pooling over T=4096 frames into S=256 segments
  3. FCSN: conv1d(k=3) + relu + conv1d(k=1) over segments -> keysteps [106, S]
  4. softmax over classes, keystep-weighted segment reduce -> f_keysteps
  5. keystep attention -> f_videos -> category logits

Sharding: T=4096 frames split across 8 cores (512 frames each). Each core
streams its 51.4MB feature shard once (memory-bound part), computes spatial
attention + per-segment partial sums on device, AllReduces the [256, 512]
partial segment sums, then every core redundantly computes the tiny FCSN /
attention tail; core 0's tail outputs are used.

Inner loop, per chunk of 7 regions ([128, 7, 512], 1.75MB fp32 HBM read):
  - DMA : gpsimd SWDGE cast-DMA loads X as fp16 (HBM still reads the
          full fp32 bytes, SBUF holds half) — this path also starts
          before the collective-runtime entry barrier, hiding launch skew
  - DVE : one batched fp16 multiply prod = X * att_w at 2x mode (~2us),
          a batched free-dim reduce for 4 regions, three diag builds
  - ACT : copy+accum_out dot reductions for 3 regions, batched exp,
          four diag(e) = identity * e builds
  - PE  : psum += diag(e_r) @ X_r per region (fp16 matmul, fp32 PSUM)
The loop is DMA-bound (~270GB/s on the cast path); fp16 keeps rel err
at ~4e-4 (fp16 mantissa, 11 bits, vs bf16's 8).
Segment partial sums accumulate in two pinned PSUM banks via one-hot
matmuls (float32r); the [256, 512] partials are AllReduced across the
8 cores and the FCSN tail runs redundantly on every core in float32r
(producers round via bitcast).
Softmax is computed without max-subtraction: s = feature . att_w has
sigma ~ 0.45, so exp() is numerically safe; the additive att_b/att2_b
biases are softmax-invariant and dropped.
"""

import os

os.environ.setdefault("MYCRO_LOCAL_CACHE", "1")

import numpy as np

N_CORES = 8
T, R, D = 4096, 49, 512
TL = T // N_CORES          # frames per core = 512
NG = TL // 128             # groups of 128 frames = 4
RC = 7                     # regions per DMA chunk (49 = 7*7)
NCH = R // RC              # chunks per group = 7
NSEG = 256
NCLS, NCAT = 106, 18
F32 = np.float32

_BUILT = None  # cached (nc, out_names)


def _build_bass():
    """Build the Bass module (single SPMD program for all 8 cores)."""
    import concourse.bacc as bacc
    import concourse.mybir as mybir
    import concourse.tile as tile

    f32 = mybir.dt.float32
    f32r = mybir.dt.float32r
    bf16 = mybir.dt.float16
    Alu = mybir.AluOpType
    Act = mybir.ActivationFunctionType
    Ax = mybir.AxisListType

    nc = bacc.Bacc(
        "TRN2",
        debug=False,
        enable_asserts=False,
        target_bir_lowering=False,
        num_devices=N_CORES,
    )

    # ---- external inputs ----
    feat = nc.dram_tensor("feat", [TL, R, D], f32, kind="ExternalInput").ap()
    idsf = nc.dram_tensor("idsf", [NG, 128, 1], f32, kind="ExternalInput").ap()
    invc = nc.dram_tensor("invc", [128, 2], f32, kind="ExternalInput").ap()
    wbatt = nc.dram_tensor("wbatt", [128, D], f32, kind="ExternalInput").ap()
    iota = nc.dram_tensor("iota", [128, NSEG], f32, kind="ExternalInput").ap()
    ident = nc.dram_tensor("ident", [128, 128], f32, kind="ExternalInput").ap()
    w1t = nc.dram_tensor("w1t", [12, 128, D], f32, kind="ExternalInput").ap()
    b1p = nc.dram_tensor("b1p", [128, 4], f32, kind="ExternalInput").ap()
    w2t = nc.dram_tensor("w2t", [4, 128, NCLS], f32, kind="ExternalInput").ap()
    b2r = nc.dram_tensor("b2r", [128, NCLS], f32, kind="ExternalInput").ap()
    att2p = nc.dram_tensor("att2p", [128, 4], f32, kind="ExternalInput").ap()
    clswp = nc.dram_tensor("clswp", [128, 4 * NCAT], f32, kind="ExternalInput").ap()
    clsb = nc.dram_tensor("clsb", [1, NCAT], f32, kind="ExternalInput").ap()
    ones11 = nc.dram_tensor("ones11", [1, 1], f32, kind="ExternalInput").ap()

    # ---- external outputs ----
    alpha_out = nc.dram_tensor("alpha_out", [TL, R], f32, kind="ExternalOutput").ap()
    ksT_out = nc.dram_tensor("ksT_out", [NSEG, NCLS], f32, kind="ExternalOutput").ap()
    cats_out = nc.dram_tensor("cats_out", [1, NCAT], f32, kind="ExternalOutput").ap()
    alpha2_out = nc.dram_tensor("alpha2_out", [1, NCLS], f32, kind="ExternalOutput").ap()

    with tile.TileContext(nc) as tc:
        with (
            tc.tile_pool(name="const", bufs=1) as cpool,
            tc.tile_pool(name="xs", bufs=6) as xpool,
            tc.tile_pool(name="work", bufs=2) as work,
            tc.tile_pool(name="prodp", bufs=4) as prodp,
            tc.tile_pool(name="diagp", bufs=10) as diagp,
            tc.tile_pool(name="small", bufs=5) as small,
            tc.tile_pool(name="mainps", bufs=2, space="PSUM") as mainps,
            tc.tile_pool(name="segps", bufs=1, space="PSUM") as segps,
            tc.tile_pool(name="dram", bufs=1, space="DRAM") as dram,
        ):
            # ---- main-loop constants into SBUF (tail weights loaded later) ----
            wb7 = cpool.tile([128, RC, D], bf16)
            for j in range(RC):
                nc.gpsimd.dma_start(wb7[:, j, :], wbatt)
            iot = cpool.tile([128, NSEG], f32)
            nc.gpsimd.dma_start(iot[:], iota)
            idn = cpool.tile([128, 128], f32)
            nc.gpsimd.dma_start(idn[:], ident)
            idnh = cpool.tile([128, 128], bf16)
            nc.gpsimd.dma_start(idnh[:], ident)

            # ---- pinned per-core segment-sum accumulators (PSUM) ----
            segA = segps.tile([128, D], f32, name="segA")
            segB = segps.tile([128, D], f32, name="segB")

            # ================= main loop: spatial attention =================
            for g in range(NG):
                nump = mainps.tile([128, D], f32, name="nump")
                E = small.tile([128, R], f32, name="E")
                for c in range(NCH):
                    X = xpool.tile([128, RC, D], bf16, name="X")
                    nc.gpsimd.dma_start(
                        X[:], feat[g * 128:(g + 1) * 128, c * RC:(c + 1) * RC, :]
                    )
                    # one batched multiply for all RC regions of this chunk
                    prod = prodp.tile([128, RC, D], bf16, name="prod")
                    nc.vector.tensor_tensor(
                        out=prod[:], in0=X[:], in1=wb7[:], op=Alu.mult
                    )
                    # dot reductions: rr=0..3 one batched DVE reduce, rr=4..6 ACT
                    S7 = small.tile([128, RC], f32, name="S7")
                    nc.vector.reduce_sum(S7[:, 0:4], prod[:, 0:4, :], axis=Ax.X)
                    for rr in range(4, RC):
                        scrap = work.tile([128, D], bf16, name="scrap")
                        nc.scalar.activation(
                            scrap[:], prod[:, rr, :], Act.Copy,
                            accum_out=S7[:, rr:rr + 1],
                        )
                    # batched exp for the chunk
                    nc.scalar.activation(
                        E[:, c * RC:(c + 1) * RC], S7[:], Act.Exp
                    )
                    for rr in range(RC):
                        r = c * RC + rr
                        diag = diagp.tile([128, 128], bf16, name="diag")
                        if rr < 3:
                            nc.vector.tensor_scalar_mul(
                                diag[:], idnh[:], E[:, r:r + 1]
                            )
                        else:
                            nc.scalar.activation(
                                diag[:], idnh[:], Act.Copy, scale=E[:, r:r + 1]
                            )
                        nc.tensor.matmul(
                            out=nump[:],
                            lhsT=diag[:],
                            rhs=X[:, rr, :],
                            start=(r == 0),
                            stop=(r == R - 1),
                        )
                # ---- group finalize ----
                den = small.tile([128, 1], f32, name="den")
                nc.vector.reduce_sum(den[:], E[:], axis=Ax.X)
                rec = small.tile([128, 1], f32, name="rec")
                nc.vector.reciprocal(rec[:], den[:])
                alph = small.tile([128, R], f32, name="alph")
                nc.scalar.activation(alph[:], E[:], Act.Copy, scale=rec[:])
                nc.sync.dma_start(alpha_out[g * 128:(g + 1) * 128, :], alph[:])
                fbar = work.tile([128, D], f32, name="fbar")
                nc.scalar.activation(fbar[:].bitcast(f32r), nump[:], Act.Copy, scale=rec[:])
                idc = small.tile([128, 1], f32, name="idc")
                nc.gpsimd.dma_start(idc[:], idsf[g])
                oh = work.tile([128, NSEG], f32, name="oh")
                nc.vector.tensor_scalar(
                    out=oh[:].bitcast(f32r), in0=iot[:], scalar1=idc[:], scalar2=None,
                    op0=Alu.is_equal,
                )
                nc.tensor.matmul(
                    out=segA[:], lhsT=oh[:, 0:128].bitcast(f32r), rhs=fbar[:].bitcast(f32r),
                    start=(g == 0), stop=(g == NG - 1), skip_group_check=True,
                )
                nc.tensor.matmul(
                    out=segB[:], lhsT=oh[:, 128:256].bitcast(f32r), rhs=fbar[:].bitcast(f32r),
                    start=(g == 0), stop=(g == NG - 1), skip_group_check=True,
                )

            # ============ cross-core segment-sum AllReduce ============
            segAs = work.tile([128, D], bf16, name="segAs")
            nc.scalar.activation(segAs[:], segA[:], Act.Copy)
            segBs = work.tile([128, D], bf16, name="segBs")
            nc.scalar.activation(segBs[:], segB[:], Act.Copy)
            ccin = dram.tile([NSEG, D], bf16, name="ccin")
            nc.sync.dma_start(ccin[0:128, :], segAs[:])
            nc.sync.dma_start(ccin[128:256, :], segBs[:])
            ccout = dram.tile([NSEG, D], bf16, name="ccout")
            nc.gpsimd.collective_compute(
                "AllReduce",
                Alu.add,
                replica_groups=[list(range(N_CORES))],
                ins=[ccin[:].opt()],
                outs=[ccout[:].opt()],
            )

            # ---- tail weights: load + f32r-round (overlaps the collective) ----
            w1sb = cpool.tile([128, 12, D], f32)
            for kc in range(12):
                wslot = work.tile([128, D], f32, name="wslot")
                nc.gpsimd.dma_start(wslot[:], w1t[kc])
                nc.scalar.activation(
                    w1sb[:, kc, :].bitcast(f32r), wslot[:], Act.Copy
                )
            b1sb = cpool.tile([128, 4], f32)
            nc.gpsimd.dma_start(b1sb[:], b1p)
            w2sb = cpool.tile([128, 4, NCLS], f32)
            for ic in range(4):
                wslot = work.tile([128, D], f32, name="wslot")
                nc.gpsimd.dma_start(wslot[:, 0:NCLS], w2t[ic])
                nc.scalar.activation(
                    w2sb[:, ic, :].bitcast(f32r), wslot[:, 0:NCLS], Act.Copy
                )
            b2sb = cpool.tile([128, NCLS], f32)
            nc.gpsimd.dma_start(b2sb[:], b2r)
            a2sb = cpool.tile([128, 4], f32)
            nc.gpsimd.dma_start(a2sb[:], att2p)
            clssb = cpool.tile([128, 4 * NCAT], f32)
            nc.gpsimd.dma_start(clssb[:], clswp)
            clsbsb = cpool.tile([1, NCAT], f32)
            nc.gpsimd.dma_start(clsbsb[:], clsb)
            onesb = cpool.tile([1, 1], f32)
            nc.gpsimd.dma_start(onesb[:], ones11)
            invsb = cpool.tile([128, 2], f32)
            nc.gpsimd.dma_start(invsb[:], invc)



            # ============ tail: segment means + FCSN + attention ============
            # fseg[s, d] (segment means), two 128-seg tiles
            fsegs = []
            for j in range(2):
                raw = work.tile([128, D], bf16, name="fsraw")
                nc.sync.dma_start(raw[:], ccout[j * 128:(j + 1) * 128, :])
                fs = cpool.tile([128, D], f32, name=f"fseg{j}", tag=f"fseg{j}")
                nc.scalar.activation(fs[:].bitcast(f32r), raw[:], Act.Copy, scale=invsb[:, j:j + 1])
                fsegs.append(fs)

            # fsegT[d, s] with one zero column of padding on each side
            fsegT = []
            for jd in range(4):
                ft = cpool.tile([128, NSEG + 2], f32, name=f"fsegT{jd}", tag=f"fsegT{jd}")
                nc.scalar.activation(ft[:, 0:1].bitcast(f32r), idn[:, 0:1], Act.Copy, scale=0.0)
                nc.scalar.activation(ft[:, NSEG + 1:NSEG + 2].bitcast(f32r), idn[:, 0:1], Act.Copy, scale=0.0)
                for j in range(2):
                    tp = mainps.tile([128, 128], f32, name="tp")
                    nc.tensor.transpose(
                        tp[:], fsegs[j][:, jd * 128:(jd + 1) * 128], idn[:]
                    )
                    nc.scalar.activation(
                        ft[:, 1 + j * 128:1 + (j + 1) * 128].bitcast(f32r),
                        tp[:], Act.Copy
                    )
                fsegT.append(ft)

            # conv1 (k=3, SAME) + relu -> h[o, s], 4 o-tiles
            hts = []
            for ot in range(4):
                hps = mainps.tile([128, NSEG], f32, name="hps")
                first = True
                for k in range(3):
                    for icn in range(4):
                        nc.tensor.matmul(
                            out=hps[:],
                            lhsT=w1sb[:, k * 4 + icn, ot * 128:(ot + 1) * 128].bitcast(f32r),
                            rhs=fsegT[icn][:, k:k + NSEG].bitcast(f32r),
                            start=first,
                            stop=(k == 2 and icn == 3),
                        )
                        first = False
                ht = cpool.tile([128, NSEG], f32, name=f"h{ot}", tag=f"h{ot}")
                nc.scalar.activation(
                    ht[:].bitcast(f32r), hps[:], Act.Relu, bias=b1sb[:, ot:ot + 1]
                )
                hts.append(ht)

            # conv2 (k=1) -> keysteps^T [s, m]; + bias; softmax over m
            attT = []
            for st in range(2):
                kps = mainps.tile([128, NCLS], f32, name="kps")
                for icn in range(4):
                    nc.tensor.matmul(
                        out=kps[:],
                        lhsT=hts[icn][:, st * 128:(st + 1) * 128].bitcast(f32r),
                        rhs=w2sb[:, icn, :].bitcast(f32r),
                        start=(icn == 0),
                        stop=(icn == 3),
                    )
                ksT = work.tile([128, NCLS], f32, name="ksT")
                nc.vector.tensor_tensor(
                    out=ksT[:], in0=kps[:], in1=b2sb[:], op=Alu.add
                )
                nc.sync.dma_start(ksT_out[st * 128:(st + 1) * 128, :], ksT[:])
                mx = small.tile([128, 1], f32, name="mx")
                nc.vector.reduce_max(mx[:], ksT[:], axis=Ax.X)
                nmx = small.tile([128, 1], f32, name="nmx")
                nc.scalar.activation(nmx[:], mx[:], Act.Copy, scale=-1.0)
                ex = work.tile([128, NCLS], f32, name="ex")
                nc.scalar.activation(ex[:], ksT[:], Act.Exp, bias=nmx[:])
                sm = small.tile([128, 1], f32, name="sm")
                nc.vector.reduce_sum(sm[:], ex[:], axis=Ax.X)
                rc2 = small.tile([128, 1], f32, name="rc2")
                nc.vector.reciprocal(rc2[:], sm[:])
                at = cpool.tile([128, NCLS], f32, name=f"attT{st}", tag=f"attT{st}")
                nc.scalar.activation(at[:].bitcast(f32r), ex[:], Act.Copy, scale=rc2[:])
                attT.append(at)

            # f_keysteps [m, d] and [d, m]
            fkps = mainps.tile([NCLS, D], f32, name="fkps")
            for st in range(2):
                nc.tensor.matmul(
                    out=fkps[:], lhsT=attT[st][:].bitcast(f32r), rhs=fsegs[st][:].bitcast(f32r),
                    start=(st == 0), stop=(st == 1),
                )
            fk = cpool.tile([NCLS, D], f32, name="fk", tag="fk")
            nc.scalar.activation(fk[:], fkps[:], Act.Copy)
            fkT = []
            for jd in range(4):
                fkTps = mainps.tile([128, NCLS], f32, name="fkTps")
                for st in range(2):
                    nc.tensor.matmul(
                        out=fkTps[:],
                        lhsT=fsegs[st][:, jd * 128:(jd + 1) * 128].bitcast(f32r),
                        rhs=attT[st][:].bitcast(f32r),
                        start=(st == 0),
                        stop=(st == 1),
                    )
                fkt = cpool.tile([128, NCLS], f32, name=f"fkT{jd}", tag=f"fkT{jd}")
                nc.scalar.activation(fkt[:], fkTps[:], Act.Copy)
                fkT.append(fkt)

            # s2 = f_keysteps @ att2_w -> [1, m]; softmax over m
            s2ps = mainps.tile([1, NCLS], f32, name="s2ps")
            for jd in range(4):
                nc.tensor.matmul(
                    out=s2ps[:], lhsT=a2sb[:, jd:jd + 1], rhs=fkT[jd][:],
                    start=(jd == 0), stop=(jd == 3),
                )
            mx2 = small.tile([1, 1], f32, name="mx2")
            nc.vector.reduce_max(mx2[:], s2ps[:], axis=Ax.X)
            nmx2 = small.tile([1, 1], f32, name="nmx2")
            nc.scalar.activation(nmx2[:], mx2[:], Act.Copy, scale=-1.0)
            e2 = small.tile([1, NCLS], f32, name="e2")
            nc.scalar.activation(e2[:], s2ps[:], Act.Exp, bias=nmx2[:])
            sm2 = small.tile([1, 1], f32, name="sm2")
            nc.vector.reduce_sum(sm2[:], e2[:], axis=Ax.X)
            rc3 = small.tile([1, 1], f32, name="rc3")
            nc.vector.reciprocal(rc3[:], sm2[:])
            a2n = small.tile([1, NCLS], f32, name="a2n")
            nc.scalar.activation(a2n[:], e2[:], Act.Copy, scale=rc3[:])
            nc.sync.dma_start(alpha2_out[:], a2n[:])

            # alpha2 as a column via PE (contract dim 1)
            a2cps = mainps.tile([NCLS, 1], f32, name="a2cps")
            nc.tensor.matmul(out=a2cps[:], lhsT=a2n[:], rhs=onesb[:])
            a2c = small.tile([NCLS, 1], f32, name="a2c")
            nc.scalar.activation(a2c[:], a2cps[:], Act.Copy)

            # f_videos[d] = sum_m f_keysteps[m, d] * alpha2[m]
            fv = cpool.tile([128, 4], f32, name="fv", tag="fv")
            for jd in range(4):
                fvps = mainps.tile([128, 1], f32, name="fvps")
                nc.tensor.matmul(
                    out=fvps[:], lhsT=fk[:, jd * 128:(jd + 1) * 128], rhs=a2c[:]
                )
                nc.scalar.activation(fv[:, jd:jd + 1], fvps[:], Act.Copy)

            # cats = f_videos @ cls_w + cls_b
            cps = mainps.tile([1, NCAT], f32, name="cps")
            for jd in range(4):
                nc.tensor.matmul(
                    out=cps[:],
                    lhsT=fv[:, jd:jd + 1],
                    rhs=clssb[:, jd * NCAT:(jd + 1) * NCAT],
                    start=(jd == 0),
                    stop=(jd == 3),
                )
            catsb = small.tile([1, NCAT], f32, name="catsb")
            nc.vector.tensor_tensor(
                out=catsb[:], in0=cps[:], in1=clsbsb[:], op=Alu.add
            )
            nc.sync.dma_start(cats_out[:], catsb[:])

    nc.finalize()
    return nc


def _get_bass():
    global _BUILT
    if _BUILT is None:
        _BUILT = _build_bass()
    return _BUILT


def _make_in_maps(inputs):
    feature = np.asarray(inputs["feature"], dtype=F32)
    seg_ids = np.asarray(inputs["seg_ids"])
    att_w = np.asarray(inputs["att_w"], dtype=F32)
    att2_w = np.asarray(inputs["att2_w"], dtype=F32)
    fcsn_w1 = np.asarray(inputs["fcsn_w1"], dtype=F32)
    fcsn_b1 = np.asarray(inputs["fcsn_b1"], dtype=F32)
    fcsn_w2 = np.asarray(inputs["fcsn_w2"], dtype=F32)
    fcsn_b2 = np.asarray(inputs["fcsn_b2"], dtype=F32)
    cls_w = np.asarray(inputs["cls_w"], dtype=F32)
    cls_b = np.asarray(inputs["cls_b"], dtype=F32)

    ids = seg_ids.reshape(T).astype(np.int64)
    cnt = np.bincount(ids, minlength=NSEG)[:NSEG]
    invcnt = (1.0 / np.maximum(cnt, 1)).astype(F32)

    shared = {
        "invc": np.ascontiguousarray(invcnt.reshape(2, 128).T),
        "wbatt": np.ascontiguousarray(np.tile(att_w[:, 0], (128, 1))),
        "iota": np.ascontiguousarray(
            np.tile(np.arange(NSEG, dtype=F32), (128, 1))
        ),
        "ident": np.eye(128, dtype=F32),
        # w1t[k*4+ic] = fcsn_w1[:, ic*128:(ic+1)*128, k].T  ([i, o] layout)
        "w1t": np.ascontiguousarray(
            fcsn_w1.transpose(2, 1, 0).reshape(3, 4, 128, D).reshape(12, 128, D)
        ),
        "b1p": np.ascontiguousarray(fcsn_b1.reshape(4, 128).T),
        "w2t": np.ascontiguousarray(fcsn_w2[:, :, 0].T.reshape(4, 128, NCLS)),
        "b2r": np.ascontiguousarray(np.tile(fcsn_b2, (128, 1))),
        "att2p": np.ascontiguousarray(att2_w[:, 0].reshape(4, 128).T),
        "clswp": np.ascontiguousarray(
            cls_w.reshape(4, 128, NCAT).transpose(1, 0, 2).reshape(128, 4 * NCAT)
        ),
        "clsb": np.ascontiguousarray(cls_b.reshape(1, NCAT)),
        "ones11": np.ones((1, 1), dtype=F32),
    }
    in_maps = []
    for cix in range(N_CORES):
        m = dict(shared)
        m["feat"] = np.ascontiguousarray(feature[0, cix * TL:(cix + 1) * TL])
        m["idsf"] = np.ascontiguousarray(
            ids[cix * TL:(cix + 1) * TL].astype(F32).reshape(NG, 128, 1)
        )
        in_maps.append(m)
    return in_maps


def _assemble(results):
    alphas_sp = np.concatenate(
        [results[c]["alpha_out"] for c in range(N_CORES)], axis=0
    ).reshape(1, T, R, 1)
    keysteps = np.ascontiguousarray(results[0]["ksT_out"].T).reshape(1, NCLS, NSEG)
    cats = results[0]["cats_out"].reshape(1, NCAT)
    alphas_ks = results[0]["alpha2_out"].reshape(1, NCLS, 1)
    return (
        keysteps.astype(F32),
        cats.astype(F32),
        alphas_sp.astype(F32),
        alphas_ks.astype(F32),
    )


def run_kernel(inputs, trace=False, trace_kwargs=None):
    """Run on 8 NeuronCores. Returns (outputs_tuple, BassKernelResults)."""
    from concourse.bass_utils import run_bass_kernel_spmd

    nc = _get_bass()
    in_maps = _make_in_maps(inputs)
    kw = {}
    if trace:
        kw["trace"] = True
        if trace_kwargs:
            kw.update(trace_kwargs)
    rr = run_bass_kernel_spmd(nc, in_maps, core_ids=list(range(N_CORES)), **kw)
    return _assemble(rr.results), rr


def kernel(**inputs):
    outs, _ = run_kernel(inputs, trace=False)
    return outs
